# revision 81
# baseline (speedup 1.0000x reference)
"""Multi-head attention block (B=4, S=2048, D=1024, H=16) on 8 TRN2 cores.

Sharding: data-parallel over batch (4 batches x 2 cores) and tensor-parallel
over heads (8 heads per core).  Each core computes, for its (batch,
head-group): Q^T/K^T (head-dim-major) and V (seq-major) projections, causal
attention (scores transposed: S^T[k,q] = K Q^T, exp without max-subtraction,
row-sum via an appended ones-column in the PV matmul), context, and a partial
output projection with its w_o column slice.  The host sums the two partial
outputs per batch (the "all-reduce after w_o") and adds b_o.

Pipeline layout (v2): the Q/K projections for pair p+1 are emitted
interleaved with the attention inner loop for pair p, so the PE has
projection matmuls to execute while the ACT engine works through the exp
backlog (exp is the attention-phase serial driver otherwise).  Causal
trimming shrinks diagonal score blocks to their live columns, exp
instructions are batched two k-blocks at a time from a 2-bank PSUM tile,
the causal mask is a 128x128 triangle multiply on DVE, PV sub-blocks are
scheduled as fillers inside the next chunk's score stream (their exp
dependencies are the freshest), the output projection is folded into the
last pair's second-head loop, and PSUM->SBUF evacuations run on ACT/DVE
(GPSIMD cannot read PSUM).  All input loads ride the SP DMA queue in
first-use order; partial outputs ship as bf16.

Matmuls run in bf16 (1 PE cycle/row); accumulation is fp32 in PSUM.
kernel(**inputs) takes full unsharded inputs and returns the full output.
"""

from collections import deque

import numpy as np

import concourse.bass as bass
import concourse.mybir as mybir
import concourse.tile as tile
from concourse import bacc
from concourse.bass_utils import run_bass_kernel_spmd
from concourse.masks import make_identity

B, S, D, H = 4, 2048, 1024, 16
DK = D // H            # 64 head dim
P = 128                # partitions
NCORES = 8
HPC = H // 2           # 8 heads per core
DPC = HPC * DK         # 512 projected dims per core
NPAIR = DPC // P       # 4 head-pairs per core
KT = D // P            # 8 contraction tiles for projections
SC_W = 512             # projection seq chunk width
N_SC = S // SC_W
QC_W = 512             # query chunk width
N_QC = S // QC_W
NQB = QC_W // P        # 4 query sub-blocks per chunk
NKB = S // P           # 16 key blocks
F32 = mybir.dt.float32
BF16 = mybir.dt.bfloat16

_NC_CACHE: dict = {}


def _build_nc(causal: bool, reps: int = 1, mmdt: str = "bf16", phases: int = 3) -> bass.Bass:
    mm_dt = BF16
    in_dt = BF16

    nc = bacc.Bacc(
        "TRN2",
        debug=False,
        enable_asserts=False,
        target_bir_lowering=False,
        num_devices=NCORES,
    )

    qT = nc.dram_tensor("qT", [D, S], in_dt, kind="ExternalInput").ap()
    kT = nc.dram_tensor("kT", [D, S], in_dt, kind="ExternalInput").ap()
    vT = nc.dram_tensor("vT", [D, S], in_dt, kind="ExternalInput").ap()
    wqT = nc.dram_tensor("wqT", [D, DPC], in_dt, kind="ExternalInput").ap()
    wkT = nc.dram_tensor("wkT", [D, DPC], in_dt, kind="ExternalInput").ap()
    wvT = nc.dram_tensor("wvT", [D, DPC], in_dt, kind="ExternalInput").ap()
    woT = nc.dram_tensor("woT", [DPC, D], in_dt, kind="ExternalInput").ap()
    bq = nc.dram_tensor("bq", [DPC], F32, kind="ExternalInput").ap()
    bk = nc.dram_tensor("bk", [DPC], F32, kind="ExternalInput").ap()
    bv = nc.dram_tensor("bv", [DPC], F32, kind="ExternalInput").ap()
    # Partial outputs ship as bf16 (halves the writeback; the host sums the
    # two per-batch partials in fp32, so only ~0.2% quantization is added).
    out = nc.dram_tensor("out", [S, D], BF16, kind="ExternalOutput").ap()

    from contextlib import ExitStack

    with tile.TileContext(nc) as tc, ExitStack() as octx:
        if reps > 1:
            octx.enter_context(tc.For_i(0, reps, 1))
        ctx = octx.enter_context(ExitStack())
        singles = ctx.enter_context(tc.tile_pool(name="singles", bufs=1))

        identity = singles.tile([P, P], mm_dt)
        make_identity(nc, identity)

        if causal:
            # tri[k, u] = 1.0 if u >= k else 0.0 — the causal triangle for a
            # diagonal 128x128 score block whose window starts on its own
            # diagonal (q_global = kb*128 + u, k_global = kb*128 + k).
            tri = singles.tile([P, P], mm_dt)
            nc.gpsimd.memset(tri, 1.0)
            nc.gpsimd.affine_select(
                out=tri,
                in_=tri,
                compare_op=mybir.AluOpType.is_ge,
                fill=0.0,
                base=0,
                channel_multiplier=-1,
                pattern=[[1, P]],
            )

        # DMA issue order is the startup critical path: V-projection weights
        # and the first vT chunk first, then Q/K weights and the qT/kT
        # chunks in consumption order; woT (phase 3) last.
        # Two HWDGE queues exist (SP via nc.sync, ACT via nc.scalar).  The
        # v_x stream + background bulk ride SP; the startup-critical
        # projection weights and first qT/kT chunks ride the ACT queue,
        # which is otherwise idle until the first exp.
        # First loads split into kt-halves: the V matmul kt-loop can start
        # accumulating on the first half while the second streams in.
        w_v_sb = singles.tile([P, KT, DPC], mm_dt)
        wvr = wvT.rearrange("(kt p) d -> p kt d", p=P)
        nc.sync.dma_start(w_v_sb[:, 0:2], wvr[:, 0:2])

        bq_sb = singles.tile([P, NPAIR], F32)
        bk_sb = singles.tile([P, NPAIR], F32)
        bv_sb = singles.tile([P, NPAIR], F32)
        w_q_sb = singles.tile([P, KT, DPC], mm_dt)
        w_k_sb = singles.tile([P, KT, DPC], mm_dt)
        qT_sb = singles.tile([P, KT, S], mm_dt)
        kT_sb = singles.tile([P, KT, S], mm_dt)
        woT_sb = singles.tile([P, NPAIR, D], mm_dt)

        qr = qT.rearrange("(kt p) s -> p kt s", p=P)
        kr = kT.rearrange("(kt p) s -> p kt s", p=P)

        def emit_early_loads():
            # All loads ride the SP queue in strict first-use order.  The
            # ACT queue is kept free of dma_starts: a blocked dma_start on
            # the ACT SEQ head-of-line blocks the V-evacuation copies.
            nc.sync.dma_start(w_q_sb, wqT.rearrange("(kt p) d -> p kt d", p=P))
            nc.sync.dma_start(bq_sb, bq.rearrange("(pair p) -> p pair", p=P))
            nc.sync.dma_start(bk_sb, bk.rearrange("(pair p) -> p pair", p=P))
            nc.sync.dma_start(qT_sb[:, :, 0:SC_W], qr[:, :, 0:SC_W])
            nc.sync.dma_start(kT_sb[:, :, 0:SC_W], kr[:, :, 0:SC_W])
            nc.sync.dma_start(w_k_sb, wkT.rearrange("(kt p) d -> p kt d", p=P))
            csl = slice(SC_W, 2 * SC_W)
            nc.sync.dma_start(qT_sb[:, :, csl], qr[:, :, csl])
            nc.sync.dma_start(kT_sb[:, :, csl], kr[:, :, csl])

        def emit_late_loads():
            nc.sync.dma_start(bv_sb, bv.rearrange("(pair p) -> p pair", p=P))
            for sc in range(2, N_SC):
                csl = slice(sc * SC_W, (sc + 1) * SC_W)
                nc.sync.dma_start(qT_sb[:, :, csl], qr[:, :, csl])
                nc.sync.dma_start(kT_sb[:, :, csl], kr[:, :, csl])
            nc.sync.dma_start(
                woT_sb, woT.rearrange("(pair p) dm -> p pair dm", p=P))

        # Persistent activations
        QT_cur = singles.tile([P, 2, S], mm_dt)   # [d%64(hp), buf, s]
        KT_cur = singles.tile([P, 2, S], mm_dt)
        V_all = singles.tile([P, NKB, HPC, DK + 1], mm_dt)  # [s%128, kb, h, d|1]
        ctxT_all = singles.tile([P, NPAIR, S], mm_dt)
        nc.vector.memset(V_all[:, :, :, DK:DK + 1], 1.0)

        with (
            tc.tile_pool(name="vstage", bufs=2) as vst,
            tc.tile_pool(name="ptpool", bufs=17) as ptp,
            tc.tile_pool(name="little", bufs=8) as lit,
            tc.tile_pool(name="stage", bufs=2) as stg,
            tc.tile_pool(name="ostage", bufs=4) as ost,
            tc.tile_pool(name="qkpsum", bufs=2, space="PSUM") as qkp,
            tc.tile_pool(name="spsum", bufs=2, space="PSUM") as sp,
            tc.tile_pool(name="cpsum", bufs=2, space="PSUM") as cp,
        ):
            # ---- Prologue: V projection chunks 0-1, pair-0 projections for
            # chunk 0, then into the pair loop; V chunks 2-3 and the
            # remaining pair-0 projections ride along as pair-0 fillers so
            # attention (and with it the ACT exp stream) starts as soon as
            # the first projected chunk lands.
            def dma_v_x(sc, split=False):
                v_x = vst.tile([P, KT, SC_W], in_dt, name="v_x")
                vr = vT.rearrange("(kt p) s -> p kt s", p=P)[
                    :, :, sc * SC_W:(sc + 1) * SC_W]
                if split:
                    nc.sync.dma_start(v_x[:, 0:2], vr[:, 0:2])
                    nc.sync.dma_start(w_v_sb[:, 2:4], wvr[:, 2:4])
                    nc.sync.dma_start(v_x[:, 2:4], vr[:, 2:4])
                    nc.sync.dma_start(w_v_sb[:, 4:KT], wvr[:, 4:KT])
                    nc.sync.dma_start(v_x[:, 4:KT], vr[:, 4:KT])
                else:
                    nc.sync.dma_start(v_x, vr)
                return v_x

            def emit_v_chunk(sc, v_x):
                for ss in range(SC_W // P):
                    ps = qkp.tile([P, DPC], F32, name="ps_v", tag="ps_qk")
                    for kt in range(KT):
                        nc.tensor.matmul(
                            ps,
                            lhsT=v_x[:, kt, ss * P:(ss + 1) * P],
                            rhs=w_v_sb[:, kt, :],
                            start=(kt == 0),
                            stop=(kt == KT - 1),
                        )
                    sblk = sc * (SC_W // P) + ss
                    # GPSIMD cannot read PSUM on TRN2; ACT is idle here.
                    nc.scalar.copy(
                        out=V_all[:, sblk, :, 0:DK],
                        in_=ps.rearrange("p (h d) -> p h d", h=HPC),
                    )

            def emit_proj_chunk(pair, which, sc):
                """Project one 512-seq chunk of Q^T or K^T for `pair`."""
                w_sb = w_q_sb if which == 0 else w_k_sb
                x_sb = qT_sb if which == 0 else kT_sb
                bias = bq_sb if which == 0 else bk_sb
                dest = QT_cur if which == 0 else KT_cur
                ps = qkp.tile([P, SC_W], F32, name="ps_qk", tag="ps_qk")
                for kt in range(KT):
                    nc.tensor.matmul(
                        ps,
                        lhsT=w_sb[:, kt, pair * P:(pair + 1) * P],
                        rhs=x_sb[:, kt, sc * SC_W:(sc + 1) * SC_W],
                        start=(kt == 0),
                        stop=(kt == KT - 1),
                    )
                nc.vector.tensor_scalar_add(
                    out=dest[:, pair % 2, sc * SC_W:(sc + 1) * SC_W],
                    in0=ps,
                    scalar1=bias[:, pair:pair + 1],
                )

            # Prologue emission
            v_x0, v_x1 = dma_v_x(0, split=True), dma_v_x(1)
            emit_early_loads()
            emit_v_chunk(0, v_x0)
            v_x2 = dma_v_x(2)
            emit_v_chunk(1, v_x1)
            v_x3 = dma_v_x(3)
            emit_late_loads()
            emit_proj_chunk(0, 0, 0)
            emit_proj_chunk(0, 1, 0)

            # Pair-0 hp0 filler seed: remaining V chunks + pair-0
            # projections, ordered by need (scores(j) needs proj chunk j;
            # PV(j) needs V chunk j).
            slot_seed = {
                0: [lambda: emit_proj_chunk(0, 0, 1),
                    lambda: emit_proj_chunk(0, 1, 1)],
                1: [lambda: emit_v_chunk(2, v_x2),
                    lambda: emit_proj_chunk(0, 0, 2),
                    lambda: emit_proj_chunk(0, 1, 2)],
                2: [lambda: emit_v_chunk(3, v_x3),
                    lambda: emit_proj_chunk(0, 0, 3),
                    lambda: emit_proj_chunk(0, 1, 3)],
            }

            # Next-pair projection chunks per slot j: late chunks have more
            # exp backlog (ACT runs scores at half the PE streaming rate),
            # so they get more PE filler.  Pair 0's own hp0 slots are full
            # of prologue fillers, so pair 1's chunks all land in hp1.
            def proj_count(pair, hp, j):
                if pair == 0:
                    return 0 if hp == 0 else 2
                return (0, 1, 1, 2)[j]

            NDC = D // 512

            # Pair-end transposes carry over as fillers for the next pair's
            # first-head slots (pair 3's first head otherwise has no filler
            # work at all and runs ACT-bound).
            carry = []

            for pair in range(NPAIR):
                pbuf = pair % 2
                last_pair = pair == NPAIR - 1
                ctx_stage = stg.tile([P, NKB, P], mm_dt, name="ctx_stage")
                slot = 0
                inherited = carry
                carry = []

                def emit_transpose(sb, ctx_stage=ctx_stage, pair=pair):
                    # Default-bound: carried thunks run during the NEXT pair.
                    tps = cp.tile([P, P], mm_dt, name="tps", tag="cps")
                    nc.tensor.transpose(tps, ctx_stage[:, sb, :], identity)
                    nc.vector.tensor_scalar_add(
                        out=ctxT_all[:, pair, sb * P:(sb + 1) * P],
                        in0=tps,
                        scalar1=bv_sb[:, pair:pair + 1],
                    )

                def emit_p3(sb, dmc):
                    # Output projection for one 128x512 block; only legal
                    # once all pairs' ctxT at sb are final (pair-3 tail loop).
                    ps = qkp.tile([P, 512], F32, name="ps_o", tag="ps_qk")
                    for pr in range(NPAIR):
                        nc.tensor.matmul(
                            ps,
                            lhsT=ctxT_all[:, pr, sb * P:(sb + 1) * P],
                            rhs=woT_sb[:, pr, dmc * 512:(dmc + 1) * 512],
                            start=(pr == 0),
                            stop=(pr == NPAIR - 1),
                        )
                    o_sb = ost.tile([P, 512], BF16, name="o_sb")
                    # GPSIMD cannot read PSUM; alternate ACT / DVE.
                    if dmc % 2 == 0:
                        nc.scalar.copy(out=o_sb, in_=ps)
                    else:
                        nc.vector.tensor_copy(out=o_sb, in_=ps)
                    nc.sync.dma_start(
                        out[sb * P:(sb + 1) * P, dmc * 512:(dmc + 1) * 512],
                        o_sb,
                    )

                for hp in range(2):
                    psl = slice(hp * DK, (hp + 1) * DK)
                    tail = last_pair and hp == 1

                    def emit_scores(j, fillers=None, late=None):
                        """Score blocks for chunk j, causally trimmed, exp'd
                        two k-blocks per ACT instruction.  Filler thunks (PE
                        work with no exp dependency) are emitted between
                        score groups so the PE stays busy while the ACT
                        drains the score-PSUM double buffer.  Returns
                        {kb: (pt_tile, base_col, qstart)}."""
                        kb_hi = min(NKB, (j + 1) * NQB) if causal else NKB
                        kb_diag0 = j * NQB if causal else kb_hi
                        tri_by_c = {}
                        blocks = []  # (kb, qstart, width)
                        for kb in range(kb_hi):
                            if causal and kb >= kb_diag0:
                                c = kb - kb_diag0
                                qs = j * QC_W + c * P
                                w = QC_W - c * P
                            else:
                                qs = j * QC_W
                                w = QC_W
                            blocks.append((kb, qs, w))
                        # Group order: first diagonal pair FIRST (its exp
                        # gates the next chunk's first PV sub-blocks), then
                        # the off-diagonal pairs, then the last diagonal pair
                        # (its consumers, PV qq=2,3, are injected mid-scores
                        # one chunk later, giving ACT time to finish it).
                        groups = []
                        nd, dg = blocks[:kb_diag0], blocks[kb_diag0:]
                        if dg:
                            groups.append(dg[0:2])
                        groups += [nd[i:i + 2] for i in range(0, len(nd), 2)]
                        if len(dg) > 2:
                            groups.append(dg[2:4])
                        tiles = {}
                        late_at = min(4, len(groups) - 1)
                        for gi, grp in enumerate(groups):
                            if gi == late_at and late is not None:
                                late()
                            if gi >= 1 and fillers:
                                fillers.popleft()()
                            tw = sum(b[2] for b in grp)
                            ps = sp.tile([P, 1024], F32, name="ps_s", tag="ps_s")
                            pt = ptp.tile([P, 1024], mm_dt, name="pt", tag="pt")
                            col = 0
                            for (kb, qs, w) in grp:
                                nc.tensor.matmul(
                                    ps[:, col:col + w],
                                    lhsT=KT_cur[psl, pbuf, kb * P:(kb + 1) * P],
                                    rhs=QT_cur[psl, pbuf, qs:qs + w],
                                    start=True,
                                    stop=True,
                                )
                                tiles[kb] = (pt, col, qs)
                                col += w
                            nc.scalar.activation(
                                pt[:, 0:tw], ps[:, 0:tw],
                                mybir.ActivationFunctionType.Exp,
                                scale=1.0 / np.sqrt(DK),
                            )
                            # Triangle masks run lazily right before the
                            # first PV sub-block that reads them (emitting
                            # them eagerly parks them in the DVE queue while
                            # they wait out the ACT backlog).
                            for (kb, qs, w) in grp:
                                if causal and kb >= kb_diag0:
                                    c0 = tiles[kb][1]
                                    tri_by_c[kb - kb_diag0] = (
                                        lambda pt=pt, c0=c0: nc.vector.tensor_mul(
                                            pt[:, c0:c0 + P], pt[:, c0:c0 + P], tri
                                        ))
                        return tiles, tri_by_c

                    def emit_pv(j, scored, qqs):
                        tiles, tri_by_c = scored
                        h = pair * 2 + hp
                        for qq in qqs:
                            if causal and qq in tri_by_c:
                                tri_by_c.pop(qq)()
                            qb = j * NQB + qq
                            kmax = (qb + 1) if causal else NKB
                            cps = cp.tile([P, DK + 1], F32, name="cps", tag="cps")
                            for kb in range(kmax):
                                pt, base, qs = tiles[kb]
                                off = base + qb * P - qs
                                nc.tensor.matmul(
                                    cps,
                                    lhsT=pt[:, off:off + P],
                                    rhs=V_all[:, kb, h, :],
                                    start=(kb == 0),
                                    stop=(kb == kmax - 1),
                                )
                            recip = lit.tile([P, 1], F32, name="recip")
                            nc.vector.reciprocal(recip, cps[:, DK:DK + 1])
                            nc.vector.tensor_scalar_mul(
                                ctx_stage[:, qb, psl], cps[:, 0:DK], scalar1=recip
                            )

                    def pv_front(j, scored):
                        # PV qq=0,1: their deps (first-diagonal + off-diag
                        # exps) are done by the slot boundary.
                        emit_pv(j, scored, [0, 1])
                        if tail:
                            for qq in (0, 1):
                                sb = j * NQB + qq
                                emit_transpose(sb)
                                for dmc in range(NDC):
                                    pending.append(
                                        lambda sb=sb, dmc=dmc: emit_p3(sb, dmc))

                    def pv_back(j, scored):
                        # PV qq=2,3 wait on chunk j's LAST diagonal exp;
                        # injected mid-scores(j+1) where ACT has drained.
                        emit_pv(j, scored, [2, 3])
                        if tail:
                            for qq in (2, 3):
                                sb = j * NQB + qq
                                emit_transpose(sb)
                                for dmc in range(NDC):
                                    emit_p3(sb, dmc)

                    pending = deque()
                    prev = None
                    for j in range(N_QC):
                        if pair == 0 and hp == 0:
                            pending.extend(slot_seed.get(j, []))
                        if hp == 0 and inherited:
                            for _ in range(NQB):
                                if inherited:
                                    pending.append(inherited.pop(0))
                        if pair + 1 < NPAIR:
                            for _ in range(proj_count(pair, hp, j)):
                                w = (pair + 1, slot % 2, slot // 2)
                                pending.append(
                                    lambda w=w: emit_proj_chunk(w[0], w[1], w[2]))
                                slot += 1
                        late = None
                        if prev is not None:
                            late = (lambda j=j, s=prev: pv_back(j - 1, s))
                            # pv_front rides as the first mid-scores filler:
                            # the slot then opens with score matmuls (which
                            # need only older exps) instead of PV matmuls
                            # (which need the freshest ones).
                            pending.appendleft(
                                lambda j=j, s=prev: pv_front(j - 1, s))
                        cur = emit_scores(j, pending, late=late)
                        # drain leftovers
                        while pending:
                            pending.popleft()()
                        prev = cur
                    pv_front(N_QC - 1, prev)
                    pv_back(N_QC - 1, prev)
                    while pending:
                        pending.popleft()()

                if not last_pair:
                    # transpose ctx to head-major and add v-bias — carried
                    # into the next pair's first-head slots as filler
                    carry = [
                        lambda sb=sb, f=emit_transpose: f(sb)
                        for sb in range(NKB)
                    ]

    if not nc.is_finalized():
        nc.finalize()
    return nc


def _get_nc(causal: bool, reps: int = 1, **kw) -> bass.Bass:
    key = (causal, reps, tuple(sorted(kw.items())))
    if key not in _NC_CACHE:
        _NC_CACHE[key] = _build_nc(causal, reps, **kw)
    return _NC_CACHE[key]


def _make_in_maps(q, k, v, w_q, w_k, w_v, w_o, b_q, b_k, b_v, in_np=None):
    import ml_dtypes
    if in_np is None:
        in_np = ml_dtypes.bfloat16
    in_maps = []
    qb = [np.ascontiguousarray(q[b].T.astype(in_np)) for b in range(B)]
    kb = [np.ascontiguousarray(k[b].T.astype(in_np)) for b in range(B)]
    vb = [np.ascontiguousarray(v[b].T.astype(in_np)) for b in range(B)]
    for c in range(NCORES):
        b, g = divmod(c, 2)
        hsl = slice(g * DPC, (g + 1) * DPC)
        in_maps.append({
            "qT": qb[b],
            "kT": kb[b],
            "vT": vb[b],
            "wqT": np.ascontiguousarray(w_q[hsl, :].T.astype(in_np)),
            "wkT": np.ascontiguousarray(w_k[hsl, :].T.astype(in_np)),
            "wvT": np.ascontiguousarray(w_v[hsl, :].T.astype(in_np)),
            "woT": np.ascontiguousarray(w_o[:, hsl].T.astype(in_np)),
            "bq": np.ascontiguousarray(b_q[hsl]),
            "bk": np.ascontiguousarray(b_k[hsl]),
            "bv": np.ascontiguousarray(b_v[hsl]),
        })
    return in_maps


def kernel(q, k, v, mask, w_q, b_q, w_k, b_k, w_v, b_v, w_o, b_o, **run_kwargs):
    q = np.asarray(q, np.float32)
    k = np.asarray(k, np.float32)
    v = np.asarray(v, np.float32)
    w_q = np.asarray(w_q, np.float32)
    w_k = np.asarray(w_k, np.float32)
    w_v = np.asarray(w_v, np.float32)
    w_o = np.asarray(w_o, np.float32)
    b_q = np.asarray(b_q, np.float32)
    b_k = np.asarray(b_k, np.float32)
    b_v = np.asarray(b_v, np.float32)
    b_o = np.asarray(b_o, np.float32)

    mask_b = np.asarray(mask).reshape(S, S).astype(bool)
    causal = bool(np.array_equal(mask_b, np.tril(np.ones((S, S), bool))))
    if not causal:
        assert mask_b.all(), "only causal or all-ones masks are supported"

    nc = _get_nc(causal)
    in_maps = _make_in_maps(q, k, v, w_q, w_k, w_v, w_o, b_q, b_k, b_v)

    res = run_bass_kernel_spmd(nc, in_maps, core_ids=list(range(NCORES)), **run_kwargs)
    outs = [np.asarray(r["out"], dtype=np.float32) for r in res.results]
    full = np.stack(
        [outs[2 * b] + outs[2 * b + 1] + b_o[None, :] for b in range(B)]
    ).astype(np.float32)
    kernel.last_result = res
    return full


kernel.last_result = None
